# revision 28
# baseline (speedup 1.0000x reference)
"""Trainium2 Bass kernel for nn_AttentionLayer (sparse_attention).

Reference computation:
    c  = relu(gamma_j @ Wa + ba0)          # [N, 8]
    s  = (c @ h + ba1)[:, 0]               # [N]
    e  = exp(inputs * s)                   # [B, N]
    p  = e / sum(e, axis=1, keepdims=True) # softmax over N
    out = p @ gamma_j                      # [B, 8]

Key identity: out = (E @ gamma) / (E @ ones) with E = exp(inputs * s);
both numerator and denominator are contractions over N, so E is never
materialized.  N is sharded across the 8 cores; each core streams its
x^T shard once.  Per 128-row n-chunk (n on partitions, B=1024 free):

    DVE:  u  = xT * s[n]                  (per-partition scalar, fp16)
    ACT:  eT = exp(u)                     (one op per 4-5 chunks, f32)
    PE :  psum += gamma_ext[n, :].T @ eT  (gamma_ext = [gamma | 1], fp32)

The PE matmuls only occupy M=9 of the 128 array columns, so successive
(chunk, b-slice) matmuls round-robin over the four 32-column array
quadrants (tile_position col packing) into four single-bank psum
accumulators; the host sums the quadrant partials.

x^T is uploaded as fp16 (halves DMA traffic): u = x*s is small
(|u| < ~0.45), so fp16 rounding of x/u perturbs exp(u) by ~2e-4
absolute at most — measured ~8e-7 absmax-scale-relative on the final
output, the same magnitude as the fp32 reference's own rounding noise.
The contraction itself stays fp32.

Host side: computes s (tiny [N] vector), pre-transposes/pads/shards
inputs, and reduces the per-core partials (numer rows 0..7, denom row
8) into the final [B, 8] output.
"""

import numpy as np

P = 128          # SBUF partitions / contraction tile
B = 1024         # batch
N = 100000       # items
D = 8
N_CORES = 8
CPG = 14         # n-chunks per group (one x DMA per group)
GROUPS = 7       # groups per core
NCH = GROUPS * CPG           # 98 chunks of 128 rows per core
NS = NCH * P                 # 12544 rows per core
NPAD = NS * N_CORES          # 100352 padded N

_prog_cache = {}


def build_program(groups, cpg, b, num_devices, first_group_split=True):
    """Build + compile the SPMD single-core program (same on all cores)."""
    from contextlib import ExitStack

    import concourse.mybir as mybir
    import concourse.tile as tile
    from concourse import bacc

    f32 = mybir.dt.float32
    f16 = mybir.dt.float16
    nch = groups * cpg
    ns = nch * P
    nc = bacc.Bacc(
        "TRN2",
        target_bir_lowering=False,
        debug=False,
        enable_asserts=False,
        num_devices=num_devices,
    )

    assert b % 512 == 0 and b // 512 == 2, "quadrant scheme assumes B=1024"
    n_sl = 2                 # 512-wide b-slices per chunk
    n_cgrp = 4               # PE column quadrants

    xt = nc.dram_tensor("xt", [ns, b], f16, kind="ExternalInput").ap()
    ge = nc.dram_tensor("ge", [ns, 9], f32, kind="ExternalInput").ap()
    st = nc.dram_tensor("st", [P, nch], f32, kind="ExternalInput").ap()
    out = nc.dram_tensor("out", [n_cgrp, 9, 512], f32,
                         kind="ExternalOutput").ap()

    xt_r = xt.rearrange("(g c p) b -> g p c b", g=groups, c=cpg, p=P)
    ge_r = ge.rearrange("(g c p) j -> g p c j", g=groups, c=cpg, p=P)

    EXP = mybir.ActivationFunctionType.Exp

    with tile.TileContext(nc) as tc:
        with ExitStack() as ctx:
            const_pool = ctx.enter_context(tc.tile_pool(name="const", bufs=1))
            x_pool = ctx.enter_context(tc.tile_pool(name="xp", bufs=2))
            ge_pool = ctx.enter_context(tc.tile_pool(name="gep", bufs=2))
            u_pool = ctx.enter_context(tc.tile_pool(name="up", bufs=2))
            et_pool = ctx.enter_context(tc.tile_pool(name="etp", bufs=3))
            acc_pool = ctx.enter_context(
                tc.tile_pool(name="accp", bufs=1, space="PSUM")
            )
            out_pool = ctx.enter_context(tc.tile_pool(name="outp", bufs=1))

            st_t = const_pool.tile([P, nch], f32)
            nc.sync.dma_start(st_t[:], st[:])

            # one psum bank (512 f32) per PE column quadrant: the
            # start-flag matmul clears has_written for its whole bank,
            # so concurrent column groups must not share banks.
            # quadrant cg accumulates b-slice s = cg % 2.
            acc = acc_pool.tile([32 * (n_cgrp - 1) + 9, n_cgrp * 512], f32)

            # ACT units: chunks per exp op (amortizes the ~352-cycle
            # per-op overhead while keeping dependencies fine-grained).
            # group 0 ramps with small units so the first exp fires as
            # soon as the first chunk lands.
            if cpg == 14:
                units = [5, 5, 4]
                ramp_units = [1, 1, 2, 3, 4, 3]
            else:
                units = [4] * (cpg // 4) + ([cpg % 4] if cpg % 4 else [])
                ramp_units = units
            assert sum(units) == cpg and sum(ramp_units) == cpg
            max_un = max(max(units), max(ramp_units))
            for g in range(groups):
                # weights first: matmuls need ge_t, and the HWDGE ring
                # is FIFO — queueing it behind the big x loads stalls PE
                ge_t = ge_pool.tile([P, cpg, 9], f32)
                nc.sync.dma_start(ge_t[:], ge_r[g])

                g_units = ramp_units if (g == 0 and first_group_split) \
                    else units
                xt_t = x_pool.tile([P, cpg, b], f16)
                if g == 0 and first_group_split:
                    # unit-granular loads so compute ramps immediately
                    c0 = 0
                    for un in g_units:
                        nc.sync.dma_start(
                            xt_t[:, c0 : c0 + un, :],
                            xt_r[g, :, c0 : c0 + un, :],
                        )
                        c0 += un
                else:
                    # half-group loads keep the pipe fed at finer grain
                    half = cpg // 2
                    nc.sync.dma_start(xt_t[:, :half, :],
                                      xt_r[g, :, :half, :])
                    nc.sync.dma_start(xt_t[:, half:, :],
                                      xt_r[g, :, half:, :])

                c0 = 0
                for un in g_units:
                    u_t = u_pool.tile([P, max_un * b], f16)
                    for i in range(un):
                        c = c0 + i
                        gc = g * cpg + c
                        nc.vector.tensor_scalar_mul(
                            u_t[:, i * b : (i + 1) * b], xt_t[:, c, :],
                            st_t[:, gc : gc + 1],
                        )
                    et = et_pool.tile([P, max_un * b], f32)
                    nc.scalar.activation(
                        et[:, : un * b], u_t[:, : un * b], EXP
                    )

                    for i in range(un):
                        c = c0 + i
                        gc = g * cpg + c
                        for s in range(n_sl):
                            cg = (n_sl * gc + s) % n_cgrp
                            r0 = 32 * cg
                            nc.tensor.matmul(
                                acc[r0 : r0 + 9, cg * 512 : (cg + 1) * 512],
                                ge_t[:, c, :],
                                et[:, i * b + 512 * s : i * b + 512 * (s + 1)],
                                start=(gc < 2),
                                stop=(gc >= nch - 2),
                                tile_position=(0, r0),
                            )
                    c0 += un

            out_t = out_pool.tile([32 * (n_cgrp - 1) + 9, n_cgrp * 512], f32)
            for cg in range(n_cgrp):
                sl = (slice(32 * cg, 32 * cg + 9),
                      slice(cg * 512, (cg + 1) * 512))
                nc.vector.tensor_copy(out_t[sl], acc[sl])
                nc.sync.dma_start(out[cg], out_t[sl])

    nc.compile()
    return nc


def _get_program():
    key = (GROUPS, CPG, B, N_CORES)
    if key not in _prog_cache:
        _prog_cache[key] = build_program(GROUPS, CPG, B, N_CORES)
    return _prog_cache[key]


def host_prep(inputs, gamma_j, Wa, ba0, ba1, h):
    """Compute s, build padded/sharded per-core input maps."""
    inputs = np.asarray(inputs, dtype=np.float32)
    gamma_j = np.asarray(gamma_j, dtype=np.float32)
    Wa = np.asarray(Wa, dtype=np.float32)
    ba0 = np.asarray(ba0, dtype=np.float32)
    ba1 = np.asarray(ba1, dtype=np.float32)
    h = np.asarray(h, dtype=np.float32)

    c = np.maximum(gamma_j @ Wa + ba0, 0.0)
    s = (c @ h)[:, 0] + ba1[0]                      # [N] f32

    s_pad = np.zeros(NPAD, dtype=np.float32)
    s_pad[:N] = s
    ge_pad = np.zeros((NPAD, 9), dtype=np.float32)
    ge_pad[:N, :8] = gamma_j
    ge_pad[:N, 8] = 1.0                             # denominator column

    xT = inputs.T.astype(np.float16)                # [N, B]

    in_maps = []
    for i in range(N_CORES):
        lo, hi = i * NS, (i + 1) * NS
        xs = np.zeros((NS, B), dtype=np.float16)
        real = min(hi, N) - lo
        if real > 0:
            xs[:real] = xT[lo : lo + real]
        in_maps.append(
            {
                "xt": xs,
                "ge": np.ascontiguousarray(ge_pad[lo:hi]),
                "st": np.ascontiguousarray(
                    s_pad[lo:hi].reshape(NCH, P).T
                ),
            }
        )
    return in_maps


def reduce_outputs(results):
    # quadrant cg holds the partial for b-slice s = cg % 2
    total = np.zeros((9, B), dtype=np.float64)
    for r in results:
        o = r["out"].astype(np.float64)             # [4, 9, 512]
        total[:, 0:512] += o[0] + o[2]
        total[:, 512:1024] += o[1] + o[3]
    out = (total[:8, :] / total[8:9, :]).T          # [B, 8]
    return np.ascontiguousarray(out.astype(np.float32))


def run(in_maps, trace=False, trace_cores=None):
    from concourse.bass_utils import run_bass_kernel_spmd

    nc = _get_program()
    return run_bass_kernel_spmd(
        nc,
        in_maps,
        list(range(N_CORES)),
        trace=trace,
        trace_cores=trace_cores,
    )


def kernel(inputs, gamma_j, Wa, ba0, ba1, h):
    in_maps = host_prep(inputs, gamma_j, Wa, ba0, ba1, h)
    br = run(in_maps)
    return reduce_outputs(br.results)



# revision 32
# speedup vs baseline: 1.1005x; 1.1005x over previous
"""Trainium2 Bass kernel for nn_AttentionLayer (sparse_attention).

Reference computation:
    c  = relu(gamma_j @ Wa + ba0)          # [N, 8]
    s  = (c @ h + ba1)[:, 0]               # [N]
    e  = exp(inputs * s)                   # [B, N]
    p  = e / sum(e, axis=1, keepdims=True) # softmax over N
    out = p @ gamma_j                      # [B, 8]

Key identity: out = (E @ gamma) / (E @ ones) with E = exp(inputs * s);
both numerator and denominator are contractions over N, so E is never
materialized.  N is sharded across the 8 cores; each core streams its
x^T shard once.  Per 128-row n-chunk (n on partitions, B=1024 free):

    DVE:  u  = xT * s[n]                  (per-partition scalar, fp16)
    ACT:  eT = exp(u)                     (one op per 4-5 chunks, f32)
    PE :  psum += gamma_ext[n, :].T @ eT  (gamma_ext = [gamma | 1], fp32)

The PE matmuls only occupy M=9 of the 128 array columns, so successive
(chunk, b-slice) matmuls round-robin over the four 32-column array
quadrants (tile_position col packing) into four single-bank psum
accumulators; the host sums the quadrant partials.

x^T is uploaded as fp16 (halves DMA traffic): u = x*s is small
(|u| < ~0.45), so fp16 rounding of x/u perturbs exp(u) by ~2e-4
absolute at most — measured ~8e-7 absmax-scale-relative on the final
output, the same magnitude as the fp32 reference's own rounding noise.
The contraction itself stays fp32.

Host side: computes s (tiny [N] vector), pre-transposes/pads/shards
inputs, and reduces the per-core partials (numer rows 0..7, denom row
8) into the final [B, 8] output.
"""

import numpy as np

P = 128          # SBUF partitions / contraction tile
B = 1024         # batch
N = 100000       # items
D = 8
N_CORES = 8
CPG = 14         # n-chunks per group (one x DMA per group)
GROUPS = 7       # groups per core
NCH = GROUPS * CPG           # 98 chunks of 128 rows per core
NS = NCH * P                 # 12544 rows per core
NPAD = NS * N_CORES          # 100352 padded N

_prog_cache = {}


def build_program(groups, cpg, b, num_devices, first_group_split=True, x_bufs=2, par_tail=True):
    """Build + compile the SPMD single-core program (same on all cores)."""
    from contextlib import ExitStack

    import concourse.mybir as mybir
    import concourse.tile as tile
    from concourse import bacc

    f32 = mybir.dt.float32
    f16 = mybir.dt.float16
    nch = groups * cpg
    ns = nch * P
    nc = bacc.Bacc(
        "TRN2",
        target_bir_lowering=False,
        debug=False,
        enable_asserts=False,
        num_devices=num_devices,
    )

    assert b % 512 == 0 and b // 512 == 2, "quadrant scheme assumes B=1024"
    n_sl = 2                 # 512-wide b-slices per chunk
    n_cgrp = 4               # PE column quadrants

    # partition-major upload: each SBUF partition reads one contiguous
    # run per group DMA (sequential HBM streaming instead of 2KB strides)
    xt = nc.dram_tensor("xt", [P, nch, b], f16, kind="ExternalInput").ap()
    ge = nc.dram_tensor("ge", [ns, 9], f32, kind="ExternalInput").ap()
    st = nc.dram_tensor("st", [P, nch], f32, kind="ExternalInput").ap()
    out = nc.dram_tensor("out", [n_cgrp, 9, 512], f32,
                         kind="ExternalOutput").ap()

    ge_r = ge.rearrange("(g c p) j -> g p c j", g=groups, c=cpg, p=P)

    EXP = mybir.ActivationFunctionType.Exp

    with tile.TileContext(nc) as tc:
        with ExitStack() as ctx:
            const_pool = ctx.enter_context(tc.tile_pool(name="const", bufs=1))
            x_pool = ctx.enter_context(tc.tile_pool(name="xp", bufs=x_bufs))
            ge_pool = ctx.enter_context(tc.tile_pool(name="gep", bufs=2))
            u_pool = ctx.enter_context(tc.tile_pool(name="up", bufs=2))
            et_pool = ctx.enter_context(tc.tile_pool(name="etp", bufs=3))
            acc_pool = ctx.enter_context(
                tc.tile_pool(name="accp", bufs=1, space="PSUM")
            )
            out_pool = ctx.enter_context(tc.tile_pool(name="outp", bufs=1))

            st_t = const_pool.tile([P, nch], f32)
            nc.sync.dma_start(st_t[:], st[:])

            # one psum bank (512 f32) per PE column quadrant: the
            # start-flag matmul clears has_written for its whole bank,
            # so concurrent column groups must not share banks.
            # quadrant cg accumulates b-slice s = cg % 2.
            acc = acc_pool.tile([32 * (n_cgrp - 1) + 9, n_cgrp * 512], f32)

            # ACT units: chunks per exp op (amortizes the ~352-cycle
            # per-op overhead while keeping dependencies fine-grained).
            # group 0 ramps with small units so the first exp fires as
            # soon as the first chunk lands.
            if cpg == 14:
                units = [5, 5, 4]
                ramp_units = [1, 1, 2, 3, 4, 3]
            else:
                units = [4] * (cpg // 4) + ([cpg % 4] if cpg % 4 else [])
                ramp_units = units
            assert sum(units) == cpg and sum(ramp_units) == cpg
            max_un = max(max(units), max(ramp_units))
            for g in range(groups):
                # weights first: matmuls need ge_t, and the HWDGE ring
                # is FIFO — queueing it behind the big x loads stalls PE
                ge_t = ge_pool.tile([P, cpg, 9], f32)
                nc.sync.dma_start(ge_t[:], ge_r[g])

                g_units = ramp_units if (g == 0 and first_group_split) \
                    else units
                xt_t = x_pool.tile([P, cpg, b], f16)
                gc0 = g * cpg
                if g == 0 and first_group_split:
                    # unit-granular loads so compute ramps immediately
                    c0 = 0
                    for un in g_units:
                        nc.sync.dma_start(
                            xt_t[:, c0 : c0 + un, :],
                            xt[:, gc0 + c0 : gc0 + c0 + un, :],
                        )
                        c0 += un
                else:
                    # half-group loads keep the pipe fed at finer grain
                    half = cpg // 2
                    nc.sync.dma_start(xt_t[:, :half, :],
                                      xt[:, gc0 : gc0 + half, :])
                    nc.sync.dma_start(xt_t[:, half:, :],
                                      xt[:, gc0 + half : gc0 + cpg, :])

                c0 = 0
                for un in g_units:
                    u_t = u_pool.tile([P, max_un * b], f16)
                    for i in range(un):
                        c = c0 + i
                        gc = g * cpg + c
                        nc.vector.tensor_scalar_mul(
                            u_t[:, i * b : (i + 1) * b], xt_t[:, c, :],
                            st_t[:, gc : gc + 1],
                        )
                    et = et_pool.tile([P, max_un * b], f32)
                    nc.scalar.activation(
                        et[:, : un * b], u_t[:, : un * b], EXP
                    )

                    for i in range(un):
                        c = c0 + i
                        gc = g * cpg + c
                        for s in range(n_sl):
                            cg = (n_sl * gc + s) % n_cgrp
                            r0 = 32 * cg
                            nc.tensor.matmul(
                                acc[r0 : r0 + 9, cg * 512 : (cg + 1) * 512],
                                ge_t[:, c, :],
                                et[:, i * b + 512 * s : i * b + 512 * (s + 1)],
                                start=(gc < 2),
                                stop=(gc >= nch - 2),
                                tile_position=(0, r0),
                            )
                    c0 += un

            out_t = out_pool.tile([32 * (n_cgrp - 1) + 9, n_cgrp * 512], f32)
            for cg in range(n_cgrp):
                sl = (slice(32 * cg, 32 * cg + 9),
                      slice(cg * 512, (cg + 1) * 512))
                if par_tail and cg % 2 == 1:
                    nc.scalar.copy(out_t[sl], acc[sl])
                    nc.scalar.dma_start(out[cg], out_t[sl])
                else:
                    nc.vector.tensor_copy(out_t[sl], acc[sl])
                    nc.sync.dma_start(out[cg], out_t[sl])

    nc.compile()
    return nc


def _get_program():
    key = (GROUPS, CPG, B, N_CORES)
    if key not in _prog_cache:
        _prog_cache[key] = build_program(GROUPS, CPG, B, N_CORES)
    return _prog_cache[key]


def host_prep(inputs, gamma_j, Wa, ba0, ba1, h):
    """Compute s, build padded/sharded per-core input maps."""
    inputs = np.asarray(inputs, dtype=np.float32)
    gamma_j = np.asarray(gamma_j, dtype=np.float32)
    Wa = np.asarray(Wa, dtype=np.float32)
    ba0 = np.asarray(ba0, dtype=np.float32)
    ba1 = np.asarray(ba1, dtype=np.float32)
    h = np.asarray(h, dtype=np.float32)

    c = np.maximum(gamma_j @ Wa + ba0, 0.0)
    s = (c @ h)[:, 0] + ba1[0]                      # [N] f32

    s_pad = np.zeros(NPAD, dtype=np.float32)
    s_pad[:N] = s
    ge_pad = np.zeros((NPAD, 9), dtype=np.float32)
    ge_pad[:N, :8] = gamma_j
    ge_pad[:N, 8] = 1.0                             # denominator column

    xT = inputs.T.astype(np.float16)                # [N, B]

    in_maps = []
    for i in range(N_CORES):
        lo, hi = i * NS, (i + 1) * NS
        xs = np.zeros((NS, B), dtype=np.float16)
        real = min(hi, N) - lo
        if real > 0:
            xs[:real] = xT[lo : lo + real]
        # partition-major swizzle: xs_sw[p, gc, :] = xs[gc*P + p, :]
        xs_sw = np.ascontiguousarray(
            xs.reshape(NCH, P, B).transpose(1, 0, 2)
        )
        in_maps.append(
            {
                "xt": xs_sw,
                "ge": np.ascontiguousarray(ge_pad[lo:hi]),
                "st": np.ascontiguousarray(
                    s_pad[lo:hi].reshape(NCH, P).T
                ),
            }
        )
    return in_maps


def reduce_outputs(results):
    # quadrant cg holds the partial for b-slice s = cg % 2
    total = np.zeros((9, B), dtype=np.float64)
    for r in results:
        o = r["out"].astype(np.float64)             # [4, 9, 512]
        total[:, 0:512] += o[0] + o[2]
        total[:, 512:1024] += o[1] + o[3]
    out = (total[:8, :] / total[8:9, :]).T          # [B, 8]
    return np.ascontiguousarray(out.astype(np.float32))


def run(in_maps, trace=False, trace_cores=None):
    from concourse.bass_utils import run_bass_kernel_spmd

    nc = _get_program()
    return run_bass_kernel_spmd(
        nc,
        in_maps,
        list(range(N_CORES)),
        trace=trace,
        trace_cores=trace_cores,
    )


def kernel(inputs, gamma_j, Wa, ba0, ba1, h):
    in_maps = host_prep(inputs, gamma_j, Wa, ba0, ba1, h)
    br = run(in_maps)
    return reduce_outputs(br.results)

